# revision 10
# baseline (speedup 1.0000x reference)
"""Trainium2 Bass kernel for ConditionalAttentionFusion-v2 (v3).

Math (per batch b, channel c, pixel y,x):
    G    = a0*rgb + a1*d + conv3x3(vars; Wc[c])          a0=Wt0*Wp0, a1=Wt0*Wp1,
                                                          Wc = Wt1*W_unc
    out  = rgb - diff*Q,   diff = rgb-d
    Q    = 1 - s*rgb + a1*diff - conv,   s = a0+a1

Sharding: pure data parallel over 8 cores.  The 4 images x 512 rows are cut
into 344 global row-groups of 6 rows (86/image, 4 pad rows per image bottom);
each core owns 43 consecutive groups, processed 4 at a time (10 iters x 4096
cols + 1 x 3072).  All I/O bf16 (gate 2e-2, measured ~8e-3).

DMA shape rule (HW-probed): HBM transfers spread across all 16 SDMA engines
ONLY at 128 partitions (114-partition transfers get stuck on 6 engines at
~40% bandwidth).  So every HBM stream is a [128, *] block:
  - ONE load per iter [128, 2*ng*2064] = 2.11 MB on the sync/HWDGE queue:
    partitions 0..113 = rgb | diff (m = 6c+yl), partitions 114..121 carry the
    conv var rows (2 maps x 8 halo rows x ng groups, 1032 cols each,
    x-halo included) - the "padding" partitions do useful work, so the var
    stream costs nothing extra.
  - ONE store per iter [128, ng*1024] = 1.05 MB on the scalar/HWDGE queue
    (partitions 114.. are memset junk the host ignores).
The 3 kx-shifted conv-operand replicas (vt48) are built on-chip by 6 strided
SBUF->SBUF copies on the sync queue (no HBM cost).

Per 1024-col PSUM block the PE accumulates  ps = -conv - diag(s) @ rgb
(two stationaries, K=48 and K=114; 8 matmuls/iter).  ACT drains PSUM with the
fused "+1" bias (tq = ps+1), and DVE finishes with a fused
scalar_tensor_tensor + 2 ops:  q = a1*diff + tq;  out = rgb - diff*q.
Engine budget per ~9us iter: SDMA 9.1, PE ~6.5, DVE ~6.3, ACT ~4.7.
"""
import sys

if "/opt/trn_rl_repo" not in sys.path:
    sys.path.insert(0, "/opt/trn_rl_repo")

import numpy as np

import concourse.bacc as bacc
import concourse.mybir as mybir
import concourse.tile as tile
from concourse.bass_utils import run_bass_kernel_spmd

F32 = mybir.dt.float32
BF16 = mybir.dt.bfloat16
NPBF = mybir.dt.np(BF16)
ALU = mybir.AluOpType

B, C, H, W = 4, 19, 512, 1024
YL = 6                  # rows per group
GPI = 86                # groups per image (516 rows, 4 pad)
NGRP = B * GPI          # 344 global groups
NCORES = 8
NG = NGRP // NCORES     # 43 groups per core
M = C * YL              # 114 output partitions
K = 48                  # conv contraction: 3 kx * 2 maps * 8 halo rows
VW = W + 8              # 1032 var cols per (map, halo-row, group)
LW = 2 * VW             # 2064 cols per group in the combined load
ITERS = [(4 * i, 4) for i in range(10)] + [(40, 3)]


# ----------------------------------------------------------------- host math
def _build_mats(W_prob, W_unc, W_total):
    a0 = W_total[:, 0] * W_prob[:, 0]
    a1 = W_total[:, 0] * W_prob[:, 1]
    s = a0 + a1
    Wc = W_total[:, 1][:, None, None, None] * W_unc      # [C,2,3,3]

    # ps = b48.T @ vt48 + dmat_s.T @ rgb  ==  Q - 1 - a1*diff
    b48 = np.zeros((K, M), np.float32)
    cm = np.arange(C) * YL
    for kx in range(3):
        for i in range(2):
            for r in range(8):
                for ky in range(3):
                    yl = r - ky
                    if 0 <= yl < YL:
                        b48[kx * 16 + i * 8 + r, cm + yl] = -Wc[:, i, ky, kx]

    m = np.arange(M)
    avec = np.zeros((M, 1), np.float32)
    avec[m, 0] = a1[m // YL]
    svec = np.zeros((M, 1), np.float32)
    svec[m, 0] = -s[m // YL]
    return b48.astype(NPBF), avec, svec


def _pack_groups(x, dtype=None):
    """[B, C, H, W] -> [NGRP, M, W] row-group blocks (m = 6c+yl)."""
    dtype = dtype or NPBF
    p = np.zeros((B, C, GPI * YL, W), dtype)
    p[:, :, :H] = x.astype(dtype)
    return np.ascontiguousarray(
        p.reshape(B, C, GPI, YL, W).transpose(0, 2, 1, 3, 4)
        .reshape(NGRP, M, W))


def _pack_vars(rgb_var, d_var):
    """[NGRP, 8, LW] bf16: partition r = halo row 6gl-1+r; col i*VW+x' =
    map i, x = x'-1 (zero halo)."""
    vpad = np.zeros((B, 2, GPI * YL + 6, W + 2), np.float32)
    vpad[:, 0, 1:H + 1, 1:W + 1] = rgb_var[:, 0]
    vpad[:, 1, 1:H + 1, 1:W + 1] = d_var[:, 0]
    win = np.lib.stride_tricks.sliding_window_view(vpad, 8, axis=2)
    v = win[:, :, ::YL]                        # [B, 2, 86, W+2, 8]
    v = v.transpose(0, 2, 3, 1, 4)             # [B, 86, W+2, 2, 8]
    out = np.zeros((NGRP, 8, LW), NPBF)
    vv = v.reshape(NGRP, W + 2, 2, 8).transpose(0, 3, 2, 1)   # [G, 8, 2, W+2]
    out[:, :, :W + 2] = vv[:, :, 0].astype(NPBF)
    out[:, :, VW:VW + W + 2] = vv[:, :, 1].astype(NPBF)
    return out


# ------------------------------------------------------------- bass program
_CACHE = {}


def _build_program():
    nc = bacc.Bacc("TRN2", debug=False, num_devices=NCORES)
    rd_a = nc.dram_tensor("rd_a", [10, 128, 4 * LW], BF16, kind="ExternalInput").ap()
    rd_b = nc.dram_tensor("rd_b", [128, 3 * LW], BF16, kind="ExternalInput").ap()
    b48_d = nc.dram_tensor("b48", [K, M], BF16, kind="ExternalInput").ap()
    av_d = nc.dram_tensor("avec", [M, 1], F32, kind="ExternalInput").ap()
    sv_d = nc.dram_tensor("svec", [M, 1], F32, kind="ExternalInput").ap()
    out_a = nc.dram_tensor("out_a", [10, 128, 4096], BF16, kind="ExternalOutput").ap()
    out_b = nc.dram_tensor("out_b", [128, 3072], BF16, kind="ExternalOutput").ap()

    with tile.TileContext(nc) as tc:
        with (
            tc.tile_pool(name="wpool", bufs=1) as wpool,
            tc.tile_pool(name="io", bufs=3) as io,
            tc.tile_pool(name="vtp", bufs=2) as vtp,
            tc.tile_pool(name="tqp", bufs=2) as tqp,
            tc.tile_pool(name="otp", bufs=2) as otp,
            tc.tile_pool(name="psum", bufs=2, space="PSUM") as psum,
        ):
            b48_sb = wpool.tile([K, M], BF16, name="b48_sb")
            nc.gpsimd.dma_start(out=b48_sb[:], in_=b48_d[:])
            av_sb = wpool.tile([M, 1], F32, name="av_sb")
            nc.gpsimd.dma_start(out=av_sb[:], in_=av_d[:])
            sv_sb = wpool.tile([M, 1], F32, name="sv_sb")
            nc.gpsimd.dma_start(out=sv_sb[:], in_=sv_d[:])

            for it, (j0, ng) in enumerate(ITERS):
                Wi = ng * 1024
                rdt = io.tile([128, 4 * LW], BF16, tag="rd", name=f"rd{it}")
                src = rd_a[it] if ng == 4 else rd_b[:]
                nc.sync.dma_start(out=rdt[:, :ng * LW], in_=src)
                rt = rdt[:M, 0:Wi]
                ft = rdt[:M, Wi:2 * Wi]

                # vt48[kx*16+i*8+r, g*1024+x] = rdt[114+r, g*LW+i*VW+kx+x]
                vt = vtp.tile([K, 4096], BF16, tag="vt", name=f"vt{it}")
                vsrc = rdt[114:122, :ng * LW].rearrange(
                    "p (g i x) -> p g i x", i=2, x=VW)
                for kx in range(3):
                    for i in range(2):
                        nc.sync.dma_start(
                            out=vt[16 * kx + 8 * i:16 * kx + 8 * i + 8, :Wi]
                            .rearrange("p (g x) -> p g x", x=1024),
                            in_=vsrc[:, :, i, kx:kx + 1024])

                tq = tqp.tile([M, 4096], BF16, tag="tq", name=f"tq{it}")
                for h0 in range(0, Wi, 2048):
                    hw = min(2048, Wi - h0)
                    ps = psum.tile([M, 2048], F32, tag="ps", name=f"ps{it}_{h0}")
                    for xb in range(0, hw, 512):
                        nc.tensor.matmul(
                            ps[:, xb:xb + 512], b48_sb[:, :],
                            vt[:, h0 + xb:h0 + xb + 512],
                            start=True, stop=True)
                    # tq = ps + 1  (the "1 -" of Q, fused into the drain)
                    nc.scalar.activation(
                        tq[:, h0:h0 + hw], ps[:, :hw],
                        mybir.ActivationFunctionType.Copy, bias=1.0)

                # q = a1*diff + (tq - s*rgb) ; out = rgb - diff*q
                qt = tqp.tile([M, 4096], BF16, tag="qt", name=f"qt{it}")
                nc.vector.scalar_tensor_tensor(
                    out=qt[:, :Wi], in0=ft, scalar=av_sb[:, 0:1],
                    in1=tq[:, :Wi], op0=ALU.mult, op1=ALU.add)
                q2 = tqp.tile([M, 4096], BF16, tag="q2", name=f"q2{it}")
                nc.vector.scalar_tensor_tensor(
                    out=q2[:, :Wi], in0=rt, scalar=sv_sb[:, 0:1],
                    in1=qt[:, :Wi], op0=ALU.mult, op1=ALU.add)
                pt = tqp.tile([M, 4096], BF16, tag="pt", name=f"pt{it}")
                nc.vector.tensor_mul(out=pt[:, :Wi], in0=ft, in1=q2[:, :Wi])
                ot = otp.tile([128, 4096], BF16, tag="ot", name=f"ot{it}")
                # memset base must be 32-aligned; rows 96..113 are
                # overwritten by the tensor_sub right after.
                nc.gpsimd.memset(ot[96:, :Wi], 0.0)
                nc.gpsimd.tensor_sub(out=ot[:M, :Wi], in0=rt, in1=pt[:, :Wi])
                dst = out_a[it] if ng == 4 else out_b[:]
                nc.scalar.dma_start(out=dst, in_=ot[:, :Wi])

    nc.compile()
    return nc


def _shard_inputs(rgb, d, rgb_var, d_var, W_prob, W_unc, W_total):
    rgb = np.asarray(rgb, np.float32)
    d = np.asarray(d, np.float32)
    b48, avec, svec = _build_mats(
        np.asarray(W_prob, np.float32),
        np.asarray(W_unc, np.float32),
        np.asarray(W_total, np.float32))

    pg_r = _pack_groups(rgb)
    pg_f = _pack_groups(rgb - d)
    pg_v = _pack_vars(np.asarray(rgb_var, np.float32),
                      np.asarray(d_var, np.float32))

    in_maps = []
    for core in range(NCORES):
        sl = slice(core * NG, (core + 1) * NG)
        r_c, f_c, v_c = pg_r[sl], pg_f[sl], pg_v[sl]
        rd_aa = np.zeros((10, 128, 4 * LW), NPBF)
        rd_bb = np.zeros((128, 3 * LW), NPBF)
        for it, (j0, ng) in enumerate(ITERS):
            blk = rd_aa[it] if ng == 4 else rd_bb
            Wi = ng * 1024
            blk[:M, :Wi] = (
                r_c[j0:j0 + ng].transpose(1, 0, 2).reshape(M, Wi))
            blk[:M, Wi:2 * Wi] = (
                f_c[j0:j0 + ng].transpose(1, 0, 2).reshape(M, Wi))
            blk[114:122, :ng * LW] = (
                v_c[j0:j0 + ng].transpose(1, 0, 2).reshape(8, ng * LW))
        in_maps.append({
            "rd_a": rd_aa, "rd_b": rd_bb,
            "b48": b48, "avec": avec, "svec": svec,
        })
    return in_maps


def _unshard_output(results):
    og = np.empty((NGRP, M, W), NPBF)
    for core in range(NCORES):
        oa = np.asarray(results[core]["out_a"], NPBF)
        ob = np.asarray(results[core]["out_b"], NPBF)
        g0 = core * NG
        og[g0:g0 + 40] = (
            oa[:, :M].reshape(10, M, 4, W).transpose(0, 2, 1, 3)
            .reshape(40, M, W))
        og[g0 + 40:g0 + 43] = ob[:M].reshape(M, 3, W).transpose(1, 0, 2)
    out = (og.reshape(B, GPI, C, YL, W).transpose(0, 2, 1, 3, 4)
           .reshape(B, C, GPI * YL, W)[:, :, :H].astype(np.float32))
    return np.ascontiguousarray(out)


def run(trace=False, **inputs):
    if "nc" not in _CACHE:
        _CACHE["nc"] = _build_program()
    nc = _CACHE["nc"]
    in_maps = _shard_inputs(**inputs)
    res = run_bass_kernel_spmd(nc, in_maps, list(range(NCORES)), trace=trace)
    return _unshard_output(res.results), res


def kernel(**inputs):
    out, _ = run(trace=False, **inputs)
    return out


# revision 24
# speedup vs baseline: 1.1156x; 1.1156x over previous
"""Trainium2 Bass kernel for ConditionalAttentionFusion-v2 (v3).

Math (per batch b, channel c, pixel y,x):
    G    = a0*rgb + a1*d + conv3x3(vars; Wc[c])          a0=Wt0*Wp0, a1=Wt0*Wp1,
                                                          Wc = Wt1*W_unc
    out  = rgb - diff*Q,   diff = rgb-d
    Q    = 1 - s*rgb + a1*diff - conv,   s = a0+a1

Sharding: pure data parallel over 8 cores.  The 4 images x 512 rows are cut
into 344 global row-groups of 6 rows (86/image, 4 pad rows per image bottom);
each core owns 43 consecutive groups, processed 4 at a time (10 iters x 4096
cols + 1 x 3072).  All I/O bf16 (gate 2e-2, measured ~8e-3).

DMA shape rule (HW-probed): HBM transfers spread across all 16 SDMA engines
ONLY at 128 partitions (114-partition transfers get stuck on 6 engines at
~40% bandwidth).  So every HBM stream is a [128, *] block:
  - ONE load per iter [128, 2*ng*2064] = 2.11 MB on the sync/HWDGE queue:
    partitions 0..113 = rgb | diff (m = 6c+yl), partitions 114..121 carry the
    conv var rows (2 maps x 8 halo rows x ng groups, 1032 cols each,
    x-halo included) - the "padding" partitions do useful work, so the var
    stream costs nothing extra.
  - ONE store per iter [128, ng*1024] = 1.05 MB on the scalar/HWDGE queue
    (partitions 114.. are memset junk the host ignores).
The 3 kx-shifted conv-operand replicas (vt48) are built on-chip by 6 strided
SBUF->SBUF copies on the sync queue (no HBM cost).

Per 1024-col PSUM block the PE accumulates  ps = -conv - diag(s) @ rgb
(two stationaries, K=48 and K=114; 8 matmuls/iter).  ACT drains PSUM with the
fused "+1" bias (tq = ps+1), and DVE finishes with a fused
scalar_tensor_tensor + 2 ops:  q = a1*diff + tq;  out = rgb - diff*q.
Engine budget per ~9us iter: SDMA 9.1, PE ~6.5, DVE ~6.3, ACT ~4.7.
"""
import sys

if "/opt/trn_rl_repo" not in sys.path:
    sys.path.insert(0, "/opt/trn_rl_repo")

import numpy as np

import concourse.bacc as bacc
import concourse.mybir as mybir
import concourse.tile as tile
import concourse.bass_utils as _bu
from concourse.bass_utils import run_bass_kernel_spmd

# Note: walrus's --enable-ldw-opt=true (which would dedupe the serial
# LDWEIGHTS reload between same-stationary matmuls) rejects the ldweights
# bass emits ("InstLdweights is not compatible with LDW optimization"), so
# every 512-col matmul costs ~835ns and per-channel diagonal terms are
# cheaper on ACT/DVE than as PE diag-matmuls.

F32 = mybir.dt.float32
BF16 = mybir.dt.bfloat16
NPBF = mybir.dt.np(BF16)
ALU = mybir.AluOpType

B, C, H, W = 4, 19, 512, 1024
YL = 6                  # rows per group
GPI = 86                # groups per image (516 rows, 4 pad)
NGRP = B * GPI          # 344 global groups
NCORES = 8
NG = NGRP // NCORES     # 43 groups per core
M = C * YL              # 114 output partitions
K = 48                  # conv contraction: 3 kx * 2 maps * 8 halo rows
VW = W + 8              # 1032 var cols per (map, halo-row, group)
LW = 2 * VW             # 2064 cols per group in the combined load
ITERS = [(4 * i, 4) for i in range(10)] + [(40, 3)]


# ----------------------------------------------------------------- host math
def _build_mats(W_prob, W_unc, W_total):
    a0 = W_total[:, 0] * W_prob[:, 0]
    a1 = W_total[:, 0] * W_prob[:, 1]
    s = a0 + a1
    Wc = W_total[:, 1][:, None, None, None] * W_unc      # [C,2,3,3]

    # ps = b48.T @ vt48 + dmat_s.T @ rgb  ==  Q - 1 - a1*diff
    b48 = np.zeros((K, M), np.float32)
    cm = np.arange(C) * YL
    for kx in range(3):
        for i in range(2):
            for r in range(8):
                for ky in range(3):
                    yl = r - ky
                    if 0 <= yl < YL:
                        b48[kx * 16 + i * 8 + r, cm + yl] = -Wc[:, i, ky, kx]

    m = np.arange(M)
    avec = np.zeros((M, 1), np.float32)
    avec[m, 0] = a1[m // YL]
    svec = np.zeros((M, 1), np.float32)
    svec[m, 0] = -s[m // YL]
    return b48.astype(NPBF), avec, svec


def _pack_groups(x, dtype=None):
    """[B, C, H, W] -> [NGRP, M, W] row-group blocks (m = 6c+yl)."""
    dtype = dtype or NPBF
    p = np.zeros((B, C, GPI * YL, W), dtype)
    p[:, :, :H] = x.astype(dtype)
    return np.ascontiguousarray(
        p.reshape(B, C, GPI, YL, W).transpose(0, 2, 1, 3, 4)
        .reshape(NGRP, M, W))


def _pack_vars(rgb_var, d_var):
    """[NGRP, 8, LW] bf16: partition r = halo row 6gl-1+r; col i*VW+x' =
    map i, x = x'-1 (zero halo)."""
    vpad = np.zeros((B, 2, GPI * YL + 6, W + 2), np.float32)
    vpad[:, 0, 1:H + 1, 1:W + 1] = rgb_var[:, 0]
    vpad[:, 1, 1:H + 1, 1:W + 1] = d_var[:, 0]
    win = np.lib.stride_tricks.sliding_window_view(vpad, 8, axis=2)
    v = win[:, :, ::YL]                        # [B, 2, 86, W+2, 8]
    v = v.transpose(0, 2, 3, 1, 4)             # [B, 86, W+2, 2, 8]
    out = np.zeros((NGRP, 8, LW), NPBF)
    vv = v.reshape(NGRP, W + 2, 2, 8).transpose(0, 3, 2, 1)   # [G, 8, 2, W+2]
    out[:, :, :W + 2] = vv[:, :, 0].astype(NPBF)
    out[:, :, VW:VW + W + 2] = vv[:, :, 1].astype(NPBF)
    return out


# ------------------------------------------------------------- bass program
_CACHE = {}


def _build_program():
    nc = bacc.Bacc("TRN2", debug=False, num_devices=NCORES)
    rd_a = nc.dram_tensor("rd_a", [10, 128, 4 * LW], BF16, kind="ExternalInput").ap()
    rd_b = nc.dram_tensor("rd_b", [128, 3 * LW], BF16, kind="ExternalInput").ap()
    b48_d = nc.dram_tensor("b48", [K, M], BF16, kind="ExternalInput").ap()
    av_d = nc.dram_tensor("avec", [M, 1], F32, kind="ExternalInput").ap()
    sv_d = nc.dram_tensor("svec", [M, 1], F32, kind="ExternalInput").ap()
    out_a = nc.dram_tensor("out_a", [10, 128, 4096], BF16, kind="ExternalOutput").ap()
    out_b = nc.dram_tensor("out_b", [128, 3072], BF16, kind="ExternalOutput").ap()

    with tile.TileContext(nc) as tc:
        with (
            tc.tile_pool(name="wpool", bufs=1) as wpool,
            tc.tile_pool(name="io", bufs=3) as io,
            tc.tile_pool(name="vtp", bufs=2) as vtp,
            tc.tile_pool(name="tqp", bufs=2) as tqp,
            tc.tile_pool(name="otp", bufs=2) as otp,
            tc.tile_pool(name="psum", bufs=2, space="PSUM") as psum,
        ):
            b48_sb = wpool.tile([K, M], BF16, name="b48_sb")
            nc.gpsimd.dma_start(out=b48_sb[:], in_=b48_d[:])
            av_sb = wpool.tile([M, 1], F32, name="av_sb")
            nc.gpsimd.dma_start(out=av_sb[:], in_=av_d[:])
            sv_sb = wpool.tile([M, 1], F32, name="sv_sb")
            nc.gpsimd.dma_start(out=sv_sb[:], in_=sv_d[:])

            for it, (j0, ng) in enumerate(ITERS):
                Wi = ng * 1024
                rdt = io.tile([128, 4 * LW], BF16, tag="rd", name=f"rd{it}")
                src = rd_a[it] if ng == 4 else rd_b[:]
                nc.sync.dma_start(out=rdt[:, :ng * LW], in_=src)
                rt = rdt[:M, 0:Wi]
                ft = rdt[:M, Wi:2 * Wi]

                # vt48[kx*16+i*8+r, g*1024+x] = rdt[114+r, g*LW+i*VW+kx+x]
                # Issued from gpsimd (SWDGE): these wait on the rd load, so
                # issuing them from sync would head-of-line block the next
                # iterations' loads.
                vt = vtp.tile([K, 4096], BF16, tag="vt", name=f"vt{it}")
                vsrc = rdt[114:122, :ng * LW].rearrange(
                    "p (g i x) -> p g i x", i=2, x=VW)
                for kx in range(3):
                    for i in range(2):
                        nc.gpsimd.dma_start(
                            out=vt[16 * kx + 8 * i:16 * kx + 8 * i + 8, :Wi]
                            .rearrange("p (g x) -> p g x", x=1024),
                            in_=vsrc[:, :, i, kx:kx + 1024])

                # ps = -conv ; tq = ACT drain ; fb = 1 - s*rgb on ACT
                tq = tqp.tile([M, 4096], BF16, tag="tq", name=f"tq{it}")
                for h0 in range(0, Wi, 2048):
                    hw = min(2048, Wi - h0)
                    ps = psum.tile([M, 2048], F32, tag="ps", name=f"ps{it}_{h0}")
                    for xb in range(0, hw, 512):
                        nc.tensor.matmul(
                            ps[:, xb:xb + 512], b48_sb[:, :],
                            vt[:, h0 + xb:h0 + xb + 512],
                            start=True, stop=True)
                    nc.scalar.activation(
                        tq[:, h0:h0 + hw], ps[:, :hw],
                        mybir.ActivationFunctionType.Copy)
                fb = tqp.tile([M, 4096], BF16, tag="fb", name=f"fb{it}")
                nc.scalar.activation(
                    fb[:, :Wi], rt, mybir.ActivationFunctionType.Copy,
                    scale=sv_sb[:, 0:1], bias=1.0)

                # Q = (1 - s*rgb) + a1*diff - conv ; out = rgb - diff*Q
                fa = tqp.tile([M, 4096], BF16, tag="fa", name=f"fa{it}")
                nc.vector.tensor_scalar_mul(
                    out=fa[:, :Wi], in0=ft, scalar1=av_sb[:, 0:1])
                q1 = tqp.tile([M, 4096], BF16, tag="q1", name=f"q1{it}")
                nc.vector.tensor_add(out=q1[:, :Wi], in0=fa[:, :Wi],
                                     in1=tq[:, :Wi])
                q2 = tqp.tile([M, 4096], BF16, tag="q2", name=f"q2{it}")
                nc.vector.tensor_add(out=q2[:, :Wi], in0=q1[:, :Wi],
                                     in1=fb[:, :Wi])
                pt = tqp.tile([M, 4096], BF16, tag="pt", name=f"pt{it}")
                nc.vector.tensor_mul(out=pt[:, :Wi], in0=ft, in1=q2[:, :Wi])
                ot = otp.tile([128, 4096], BF16, tag="ot", name=f"ot{it}")
                # memset base must be 32-aligned; rows 96..113 are
                # overwritten by the tensor_sub right after.
                nc.gpsimd.memset(ot[96:, :Wi], 0.0)
                nc.vector.tensor_sub(out=ot[:M, :Wi], in0=rt, in1=pt[:, :Wi])
                dst = out_a[it] if ng == 4 else out_b[:]
                nc.scalar.dma_start(out=dst, in_=ot[:, :Wi])

    nc.compile()
    return nc


def _shard_inputs(rgb, d, rgb_var, d_var, W_prob, W_unc, W_total):
    rgb = np.asarray(rgb, np.float32)
    d = np.asarray(d, np.float32)
    b48, avec, svec = _build_mats(
        np.asarray(W_prob, np.float32),
        np.asarray(W_unc, np.float32),
        np.asarray(W_total, np.float32))

    pg_r = _pack_groups(rgb)
    pg_f = _pack_groups(rgb - d)
    pg_v = _pack_vars(np.asarray(rgb_var, np.float32),
                      np.asarray(d_var, np.float32))

    in_maps = []
    for core in range(NCORES):
        sl = slice(core * NG, (core + 1) * NG)
        r_c, f_c, v_c = pg_r[sl], pg_f[sl], pg_v[sl]
        rd_aa = np.zeros((10, 128, 4 * LW), NPBF)
        rd_bb = np.zeros((128, 3 * LW), NPBF)
        for it, (j0, ng) in enumerate(ITERS):
            blk = rd_aa[it] if ng == 4 else rd_bb
            Wi = ng * 1024
            blk[:M, :Wi] = (
                r_c[j0:j0 + ng].transpose(1, 0, 2).reshape(M, Wi))
            blk[:M, Wi:2 * Wi] = (
                f_c[j0:j0 + ng].transpose(1, 0, 2).reshape(M, Wi))
            blk[114:122, :ng * LW] = (
                v_c[j0:j0 + ng].transpose(1, 0, 2).reshape(8, ng * LW))
        in_maps.append({
            "rd_a": rd_aa, "rd_b": rd_bb,
            "b48": b48, "avec": avec, "svec": svec,
        })
    return in_maps


def _unshard_output(results):
    og = np.empty((NGRP, M, W), NPBF)
    for core in range(NCORES):
        oa = np.asarray(results[core]["out_a"], NPBF)
        ob = np.asarray(results[core]["out_b"], NPBF)
        g0 = core * NG
        og[g0:g0 + 40] = (
            oa[:, :M].reshape(10, M, 4, W).transpose(0, 2, 1, 3)
            .reshape(40, M, W))
        og[g0 + 40:g0 + 43] = ob[:M].reshape(M, 3, W).transpose(1, 0, 2)
    out = (og.reshape(B, GPI, C, YL, W).transpose(0, 2, 1, 3, 4)
           .reshape(B, C, GPI * YL, W)[:, :, :H].astype(np.float32))
    return np.ascontiguousarray(out)


def run(trace=False, **inputs):
    if "nc" not in _CACHE:
        _CACHE["nc"] = _build_program()
    nc = _CACHE["nc"]
    in_maps = _shard_inputs(**inputs)
    res = run_bass_kernel_spmd(nc, in_maps, list(range(NCORES)), trace=trace)
    return _unshard_output(res.results), res


def kernel(**inputs):
    out, _ = run(trace=False, **inputs)
    return out


# revision 25
# speedup vs baseline: 1.2436x; 1.1147x over previous
"""Trainium2 Bass kernel for ConditionalAttentionFusion-v2 (v3).

Math (per batch b, channel c, pixel y,x):
    G    = a0*rgb + a1*d + conv3x3(vars; Wc[c])          a0=Wt0*Wp0, a1=Wt0*Wp1,
                                                          Wc = Wt1*W_unc
    out  = rgb - diff*Q,   diff = rgb-d
    Q    = 1 - s*rgb + a1*diff - conv,   s = a0+a1

Sharding: pure data parallel over 8 cores.  The 4 images x 512 rows are cut
into 344 global row-groups of 6 rows (86/image, 4 pad rows per image bottom);
each core owns 43 consecutive groups, processed 4 at a time (10 iters x 4096
cols + 1 x 3072).  All I/O bf16 (gate 2e-2, measured ~8e-3).

DMA shape rule (HW-probed): HBM transfers spread across all 16 SDMA engines
ONLY at 128 partitions (114-partition transfers get stuck on 6 engines at
~40% bandwidth).  So every HBM stream is a [128, *] block:
  - ONE load per iter [128, 2*ng*2064] = 2.11 MB on the sync/HWDGE queue:
    partitions 0..113 = rgb | diff (m = 6c+yl), partitions 114..121 carry the
    conv var rows (2 maps x 8 halo rows x ng groups, 1032 cols each,
    x-halo included) - the "padding" partitions do useful work, so the var
    stream costs nothing extra.
  - ONE store per iter [128, ng*1024] = 1.05 MB on the scalar/HWDGE queue
    (partitions 114.. are memset junk the host ignores).
The 3 kx-shifted conv-operand replicas (vt48) are built on-chip by 6 strided
SBUF->SBUF copies on the sync queue (no HBM cost).

Per 1024-col PSUM block the PE accumulates  ps = -conv - diag(s) @ rgb
(two stationaries, K=48 and K=114; 8 matmuls/iter).  ACT drains PSUM with the
fused "+1" bias (tq = ps+1), and DVE finishes with a fused
scalar_tensor_tensor + 2 ops:  q = a1*diff + tq;  out = rgb - diff*q.
Engine budget per ~9us iter: SDMA 9.1, PE ~6.5, DVE ~6.3, ACT ~4.7.
"""
import sys

if "/opt/trn_rl_repo" not in sys.path:
    sys.path.insert(0, "/opt/trn_rl_repo")

import numpy as np

import concourse.bacc as bacc
import concourse.mybir as mybir
import concourse.tile as tile
import concourse.bass_utils as _bu
from concourse.bass_utils import run_bass_kernel_spmd

# Note: walrus's --enable-ldw-opt=true (which would dedupe the serial
# LDWEIGHTS reload between same-stationary matmuls) rejects the ldweights
# bass emits ("InstLdweights is not compatible with LDW optimization"), so
# every 512-col matmul costs ~835ns and per-channel diagonal terms are
# cheaper on ACT/DVE than as PE diag-matmuls.

F32 = mybir.dt.float32
BF16 = mybir.dt.bfloat16
NPBF = mybir.dt.np(BF16)
ALU = mybir.AluOpType

B, C, H, W = 4, 19, 512, 1024
YL = 6                  # rows per group
GPI = 86                # groups per image (516 rows, 4 pad)
NGRP = B * GPI          # 344 global groups
NCORES = 8
NG = NGRP // NCORES     # 43 groups per core
M = C * YL              # 114 output partitions
K = 48                  # conv contraction: 3 kx * 2 maps * 8 halo rows
VW = W + 8              # 1032 var cols per (map, halo-row, group)
LW = 2 * VW             # 2064 cols per group in the combined load
ITERS = [(4 * i, 4) for i in range(10)] + [(40, 3)]


# ----------------------------------------------------------------- host math
def _build_mats(W_prob, W_unc, W_total):
    a0 = W_total[:, 0] * W_prob[:, 0]
    a1 = W_total[:, 0] * W_prob[:, 1]
    s = a0 + a1
    Wc = W_total[:, 1][:, None, None, None] * W_unc      # [C,2,3,3]

    # ps = b48.T @ vt48 + dmat_s.T @ rgb  ==  Q - 1 - a1*diff
    b48 = np.zeros((K, M), np.float32)
    cm = np.arange(C) * YL
    for kx in range(3):
        for i in range(2):
            for r in range(8):
                for ky in range(3):
                    yl = r - ky
                    if 0 <= yl < YL:
                        b48[kx * 16 + i * 8 + r, cm + yl] = -Wc[:, i, ky, kx]

    m = np.arange(M)
    avec = np.zeros((M, 1), np.float32)
    avec[m, 0] = a1[m // YL]
    svec = np.zeros((M, 1), np.float32)
    svec[m, 0] = -s[m // YL]
    return b48.astype(NPBF), avec, svec


def _pack_groups(x, dtype=None):
    """[B, C, H, W] -> [NGRP, M, W] row-group blocks (m = 6c+yl)."""
    dtype = dtype or NPBF
    p = np.zeros((B, C, GPI * YL, W), dtype)
    p[:, :, :H] = x.astype(dtype)
    return np.ascontiguousarray(
        p.reshape(B, C, GPI, YL, W).transpose(0, 2, 1, 3, 4)
        .reshape(NGRP, M, W))


def _pack_vars(rgb_var, d_var):
    """[NGRP, 8, LW] bf16: partition r = halo row 6gl-1+r; col i*VW+x' =
    map i, x = x'-1 (zero halo)."""
    vpad = np.zeros((B, 2, GPI * YL + 6, W + 2), np.float32)
    vpad[:, 0, 1:H + 1, 1:W + 1] = rgb_var[:, 0]
    vpad[:, 1, 1:H + 1, 1:W + 1] = d_var[:, 0]
    win = np.lib.stride_tricks.sliding_window_view(vpad, 8, axis=2)
    v = win[:, :, ::YL]                        # [B, 2, 86, W+2, 8]
    v = v.transpose(0, 2, 3, 1, 4)             # [B, 86, W+2, 2, 8]
    out = np.zeros((NGRP, 8, LW), NPBF)
    vv = v.reshape(NGRP, W + 2, 2, 8).transpose(0, 3, 2, 1)   # [G, 8, 2, W+2]
    out[:, :, :W + 2] = vv[:, :, 0].astype(NPBF)
    out[:, :, VW:VW + W + 2] = vv[:, :, 1].astype(NPBF)
    return out


# ------------------------------------------------------------- bass program
_CACHE = {}


def _build_program():
    nc = bacc.Bacc("TRN2", debug=False, num_devices=NCORES)
    rd_a = nc.dram_tensor("rd_a", [10, 128, 4 * LW], BF16, kind="ExternalInput").ap()
    rd_b = nc.dram_tensor("rd_b", [128, 3 * LW], BF16, kind="ExternalInput").ap()
    b48_d = nc.dram_tensor("b48", [K, M], BF16, kind="ExternalInput").ap()
    av_d = nc.dram_tensor("avec", [M, 1], F32, kind="ExternalInput").ap()
    sv_d = nc.dram_tensor("svec", [M, 1], F32, kind="ExternalInput").ap()
    out_a = nc.dram_tensor("out_a", [10, 128, 4096], BF16, kind="ExternalOutput").ap()
    out_b = nc.dram_tensor("out_b", [128, 3072], BF16, kind="ExternalOutput").ap()

    with tile.TileContext(nc) as tc:
        with (
            tc.tile_pool(name="wpool", bufs=1) as wpool,
            tc.tile_pool(name="io", bufs=3) as io,
            tc.tile_pool(name="vtp", bufs=2) as vtp,
            tc.tile_pool(name="tqp", bufs=2) as tqp,
            tc.tile_pool(name="otp", bufs=2) as otp,
            tc.tile_pool(name="psum", bufs=2, space="PSUM") as psum,
        ):
            b48_sb = wpool.tile([K, M], BF16, name="b48_sb")
            nc.gpsimd.dma_start(out=b48_sb[:], in_=b48_d[:])
            av_sb = wpool.tile([M, 1], F32, name="av_sb")
            nc.gpsimd.dma_start(out=av_sb[:], in_=av_d[:])
            sv_sb = wpool.tile([M, 1], F32, name="sv_sb")
            nc.gpsimd.dma_start(out=sv_sb[:], in_=sv_d[:])

            for it, (j0, ng) in enumerate(ITERS):
                Wi = ng * 1024
                rdt = io.tile([128, 4 * LW], BF16, tag="rd", name=f"rd{it}")
                src = rd_a[it] if ng == 4 else rd_b[:]
                nc.sync.dma_start(out=rdt[:, :ng * LW], in_=src)
                rt = rdt[:M, 0:Wi]
                ft = rdt[:M, Wi:2 * Wi]

                # vt48[kx*16+i*8+r, g*1024+x] = rdt[114+r, g*LW+i*VW+kx+x]
                # Sync keeps 3 loads in flight (io bufs), so the sem-wait
                # these have on load(it) still leaves 2 loads prefetched.
                vt = vtp.tile([K, 4096], BF16, tag="vt", name=f"vt{it}")
                vsrc = rdt[114:122, :ng * LW].rearrange(
                    "p (g i x) -> p g i x", i=2, x=VW)
                for kx in range(3):
                    for i in range(2):
                        nc.sync.dma_start(
                            out=vt[16 * kx + 8 * i:16 * kx + 8 * i + 8, :Wi]
                            .rearrange("p (g x) -> p g x", x=1024),
                            in_=vsrc[:, :, i, kx:kx + 1024])

                # load-dependent-only tail ops, overlap with conv matmuls:
                # fb = 1 - s*rgb (ACT), fa = a1*diff (DVE), q_ab = fa+fb
                fb = tqp.tile([M, 4096], BF16, tag="fb", name=f"fb{it}")
                nc.scalar.activation(
                    fb[:, :Wi], rt, mybir.ActivationFunctionType.Copy,
                    scale=sv_sb[:, 0:1], bias=1.0)
                fa = tqp.tile([M, 4096], BF16, tag="fa", name=f"fa{it}")
                nc.vector.tensor_scalar_mul(
                    out=fa[:, :Wi], in0=ft, scalar1=av_sb[:, 0:1])
                qab = tqp.tile([M, 4096], BF16, tag="qab", name=f"qab{it}")
                nc.vector.tensor_add(out=qab[:, :Wi], in0=fa[:, :Wi],
                                     in1=fb[:, :Wi])

                # per half: conv matmuls -> ACT drain -> Q -> diff*Q -> out
                ot = otp.tile([128, 4096], BF16, tag="ot", name=f"ot{it}")
                tq = tqp.tile([M, 4096], BF16, tag="tq", name=f"tq{it}")
                q2 = tqp.tile([M, 4096], BF16, tag="q2", name=f"q2{it}")
                pt = tqp.tile([M, 4096], BF16, tag="pt", name=f"pt{it}")
                for h0 in range(0, Wi, 2048):
                    hw = min(2048, Wi - h0)
                    hs = slice(h0, h0 + hw)
                    ps = psum.tile([M, 2048], F32, tag="ps", name=f"ps{it}_{h0}")
                    for xb in range(0, hw, 512):
                        nc.tensor.matmul(
                            ps[:, xb:xb + 512], b48_sb[:, :],
                            vt[:, h0 + xb:h0 + xb + 512],
                            start=True, stop=True)
                    nc.scalar.activation(
                        tq[:, hs], ps[:, :hw],
                        mybir.ActivationFunctionType.Copy)
                    nc.vector.tensor_add(out=q2[:, hs], in0=qab[:, hs],
                                         in1=tq[:, hs])
                    nc.vector.tensor_mul(out=pt[:, hs], in0=rdt[:M, Wi + h0:Wi + h0 + hw],
                                         in1=q2[:, hs])
                    nc.gpsimd.tensor_sub(out=ot[:M, hs],
                                         in0=rdt[:M, hs], in1=pt[:, hs])
                dst = out_a[it] if ng == 4 else out_b[:]
                nc.scalar.dma_start(out=dst, in_=ot[:, :Wi])

    nc.compile()
    return nc


def _shard_inputs(rgb, d, rgb_var, d_var, W_prob, W_unc, W_total):
    rgb = np.asarray(rgb, np.float32)
    d = np.asarray(d, np.float32)
    b48, avec, svec = _build_mats(
        np.asarray(W_prob, np.float32),
        np.asarray(W_unc, np.float32),
        np.asarray(W_total, np.float32))

    pg_r = _pack_groups(rgb)
    pg_f = _pack_groups(rgb - d)
    pg_v = _pack_vars(np.asarray(rgb_var, np.float32),
                      np.asarray(d_var, np.float32))

    in_maps = []
    for core in range(NCORES):
        sl = slice(core * NG, (core + 1) * NG)
        r_c, f_c, v_c = pg_r[sl], pg_f[sl], pg_v[sl]
        rd_aa = np.zeros((10, 128, 4 * LW), NPBF)
        rd_bb = np.zeros((128, 3 * LW), NPBF)
        for it, (j0, ng) in enumerate(ITERS):
            blk = rd_aa[it] if ng == 4 else rd_bb
            Wi = ng * 1024
            blk[:M, :Wi] = (
                r_c[j0:j0 + ng].transpose(1, 0, 2).reshape(M, Wi))
            blk[:M, Wi:2 * Wi] = (
                f_c[j0:j0 + ng].transpose(1, 0, 2).reshape(M, Wi))
            blk[114:122, :ng * LW] = (
                v_c[j0:j0 + ng].transpose(1, 0, 2).reshape(8, ng * LW))
        in_maps.append({
            "rd_a": rd_aa, "rd_b": rd_bb,
            "b48": b48, "avec": avec, "svec": svec,
        })
    return in_maps


def _unshard_output(results):
    og = np.empty((NGRP, M, W), NPBF)
    for core in range(NCORES):
        oa = np.asarray(results[core]["out_a"], NPBF)
        ob = np.asarray(results[core]["out_b"], NPBF)
        g0 = core * NG
        og[g0:g0 + 40] = (
            oa[:, :M].reshape(10, M, 4, W).transpose(0, 2, 1, 3)
            .reshape(40, M, W))
        og[g0 + 40:g0 + 43] = ob[:M].reshape(M, 3, W).transpose(1, 0, 2)
    out = (og.reshape(B, GPI, C, YL, W).transpose(0, 2, 1, 3, 4)
           .reshape(B, C, GPI * YL, W)[:, :, :H].astype(np.float32))
    return np.ascontiguousarray(out)


def run(trace=False, **inputs):
    if "nc" not in _CACHE:
        _CACHE["nc"] = _build_program()
    nc = _CACHE["nc"]
    in_maps = _shard_inputs(**inputs)
    res = run_bass_kernel_spmd(nc, in_maps, list(range(NCORES)), trace=trace)
    return _unshard_output(res.results), res


def kernel(**inputs):
    out, _ = run(trace=False, **inputs)
    return out


# revision 28
# speedup vs baseline: 2.1359x; 1.7175x over previous
"""Trainium2 Bass kernel for ConditionalAttentionFusion-v2.

Math (per batch b, channel c, pixel y,x):
    CD   = concat(rgb_var, d_var)                       # [2,H,W], shared
    AB   = Wp[c,0]*rgb + Wp[c,1]*d
    CDc  = conv3x3(CD, W_unc[c])                        # 2-in 1-out per channel
    G    = Wt[c,0]*AB + Wt[c,1]*CDc
    out  = rgb*G + d*(1-G) = d + (rgb-d)*G

Strategy: pure data parallel over 8 cores (core = (batch, H-half), slab of 256
rows, padded to 264 = 44 row-groups of 6 = 22 supergroups of 12).  All I/O is
bf16 (harness gate is rel_err < 2e-2; measured ~8e-3), halving HBM traffic.

Packed layout: partition m = 6*c + yl (19 channels x 6 rows = 114 partitions).
Host pre-packs every tensor so each supergroup is ONE CONTIGUOUS DRAM block
with 4 KB per-partition lines ([114, 2048] bf16 = two row-groups side by
side) — this DMA shape measurably spreads across all 16 SDMA engines, unlike
strided sources which get stuck on ~6.  The rgb/diff/var/out streams issue
from different DGE queues (sync / scalar / gpsimd) for ring-level overlap.

With Q := 1 - G and diff = rgb - d precomputed on host:

    Q[m,x]  = 1 - (a0+a1)[c]*rgb + a1[c]*diff - conv3x3(vars)   (PSUM)
    out     = rgb - diff * Q                                    (DVE, 2 ops)

Q accumulates in PSUM from 3 bf16 matmuls per 512-wide block:
  - conv: one [49,114] x [49,512] matmul; contraction partitions are
    q = (i, kx, y') — 2 var maps x 3 x-shifts x 8 y-rows (6+2 halo) — plus a
    ones-row supplying the "1 -".  Host pre-shifts var rows into var_p.
  - two diagonal matmuls apply the per-channel 1x1 coefficients to rgb/diff.
ScalarE (ACT) copies PSUM -> bf16 SBUF; VectorE runs the 2-op tail per
supergroup in 2x bf16 mode.
"""
import sys

if "/opt/trn_rl_repo" not in sys.path:
    sys.path.insert(0, "/opt/trn_rl_repo")

import numpy as np

import concourse.bacc as bacc
import concourse.mybir as mybir
import concourse.tile as tile
from concourse.bass_utils import run_bass_kernel_spmd

F32 = mybir.dt.float32
BF16 = mybir.dt.bfloat16
NPBF = mybir.dt.np(BF16)

B, C, H, W = 4, 19, 512, 1024
R = 256                # slab rows per core
RP = 264               # padded to 44 row-groups of 6
NG = RP // 6           # 44 row-groups
SG = NG // 2           # 22 supergroups (2 groups side by side in x)
YL = 6                 # rows per group
M = C * YL             # 114 output partitions per group
MP = 128               # partition-padded to 128: HWDGE spreads a DMA across
                       # all 16 SDMA engines only for ~128-partition transfers
K = 49                 # conv contraction: 2 maps * 3 kx * 8 rows + ones-row
W2 = 2 * W             # supergroup free size
NCORES = 8


# ----------------------------------------------------------------- host math
def _build_mats(W_prob, W_unc, W_total):
    a0 = W_total[:, 0] * W_prob[:, 0]          # rgb coeff of G
    a1 = W_total[:, 0] * W_prob[:, 1]          # d   coeff of G
    Wc = W_total[:, 1][:, None, None, None] * W_unc     # [C,2,3,3] conv coeff

    # Q = 1 - G with d = rgb - diff:
    #   Q = 1 - (a0+a1)*rgb + a1*diff - conv(vars)
    b49 = np.zeros((K, MP), np.float32)
    for i in range(2):
        for kx in range(3):
            for ky in range(3):
                for yl in range(YL):
                    b49[i * 24 + kx * 8 + yl + ky, yl:M:YL] = -Wc[:, i, ky, kx]
    b49[48, :] = 1.0

    dmat = np.zeros((MP, MP), np.float32)
    m = np.arange(M)
    dmat[m, m] = -(a0 + a1)[m // YL]
    avec = np.zeros((MP, 1), np.float32)
    avec[m, 0] = a1[m // YL]
    return b49.astype(NPBF), dmat.astype(NPBF), avec


def _pack_rows(slab):
    """[C, 256, W] f32 -> [22, 114, 2048] bf16; m = 6c+yl, two groups per sg."""
    p = np.zeros((C, RP, W), np.float32)
    p[:, :R] = slab
    # [c, sg, gg, yl, x] -> [sg, (c, yl), (gg, x)]
    p = p.reshape(C, SG, 2, YL, W).transpose(1, 0, 3, 2, 4).reshape(SG, M, W2)
    pp = np.zeros((SG, MP, W2), NPBF)
    pp[:, :M] = p.astype(NPBF)
    return pp


def _pack_vars(rgb_var, d_var, b, h0):
    """Shifted/replicated var rows: [22, 49, 2048] bf16, q = i*24 + kx*8 + y'."""
    vz = np.zeros((2, RP + 2, W + 2), np.float32)
    lo, hi = max(h0 - 1, 0), min(h0 + RP + 1, H)
    vz[0, lo - h0 + 1:hi - h0 + 1, 1:W + 1] = rgb_var[b, 0, lo:hi, :]
    vz[1, lo - h0 + 1:hi - h0 + 1, 1:W + 1] = d_var[b, 0, lo:hi, :]

    vp = np.empty((K, NG, W), np.float32)
    for i in range(2):
        for kx in range(3):
            s = vz[i, :, kx:kx + W]                       # [266, W]
            win = np.lib.stride_tricks.sliding_window_view(s, (8, W))
            vp[i * 24 + kx * 8:i * 24 + kx * 8 + 8] = (
                win[::YL, 0].transpose(1, 0, 2))          # [8, 44, W]
    vp[48] = 1.0
    vp = vp.reshape(K, SG, 2, W).transpose(1, 0, 2, 3).reshape(SG, K, W2)
    return np.ascontiguousarray(vp.astype(NPBF))


def _unpack_rows(out_p):
    """[22, 128, 2048] bf16 -> [C, 256, W] f32 (rows 114.. ignored)."""
    o = np.asarray(out_p, NPBF)[:, :M].reshape(
        SG, C, YL, 2, W).transpose(1, 0, 3, 2, 4)
    return o.reshape(C, RP, W)[:, :R].astype(np.float32)


# ------------------------------------------------------------- bass program
_CACHE = {}


def _build_program():
    nc = bacc.Bacc("TRN2", debug=False, num_devices=NCORES)
    rgb_p = nc.dram_tensor("rgb_p", [SG, MP, W2], BF16, kind="ExternalInput").ap()
    diff_p = nc.dram_tensor("diff_p", [SG, MP, W2], BF16, kind="ExternalInput").ap()
    var_p = nc.dram_tensor("var_p", [SG, K, W2], BF16, kind="ExternalInput").ap()
    b49 = nc.dram_tensor("b49", [K, MP], BF16, kind="ExternalInput").ap()
    dmat = nc.dram_tensor("dmat", [MP, MP], BF16, kind="ExternalInput").ap()
    avec = nc.dram_tensor("avec", [MP, 1], F32, kind="ExternalInput").ap()
    out_p = nc.dram_tensor("out_p", [SG, MP, W2], BF16, kind="ExternalOutput").ap()

    with tile.TileContext(nc) as tc:
        with (
            tc.tile_pool(name="wpool", bufs=1) as wpool,
            tc.tile_pool(name="io", bufs=7) as io,
            tc.tile_pool(name="tmp", bufs=4) as tmp,
            tc.tile_pool(name="psum", bufs=2, space="PSUM") as psum,
        ):
            b49_sb = wpool.tile([K, MP], BF16, name="b49_sb")
            nc.sync.dma_start(out=b49_sb[:], in_=b49[:])
            dmat_sb = wpool.tile([MP, MP], BF16, name="dmat_sb")
            nc.sync.dma_start(out=dmat_sb[:], in_=dmat[:])
            avec_sb = wpool.tile([MP, 1], F32, name="avec_sb")
            nc.sync.dma_start(out=avec_sb[:], in_=avec[:])

            for sg in range(SG):
                rt = io.tile([MP, W2], BF16, tag="rgb", name=f"rgb{sg}")
                nc.sync.dma_start(out=rt[:], in_=rgb_p[sg])
                ft = io.tile([MP, W2], BF16, tag="diff", name=f"diff{sg}")
                nc.scalar.dma_start(out=ft[:], in_=diff_p[sg])
                vt = io.tile([K, W2], BF16, tag="var", name=f"var{sg}")
                nc.gpsimd.dma_start(out=vt[:], in_=var_p[sg])

                ps = psum.tile([MP, W2], F32, tag="ps", name=f"ps{sg}")
                for x0 in range(0, W2, 512):
                    nc.tensor.matmul(
                        ps[:, x0:x0 + 512],
                        b49_sb[:, :],
                        vt[:, x0:x0 + 512],
                        start=True, stop=False)
                    nc.tensor.matmul(
                        ps[:, x0:x0 + 512],
                        dmat_sb[:, :],
                        rt[:, x0:x0 + 512],
                        start=False, stop=True)
                # ps holds 1 - conv - (a0+a1)*rgb; finish Q via ACT prescale
                tq = tmp.tile([MP, W2], BF16, tag="tq", name=f"tq{sg}")
                nc.scalar.copy(out=tq[:], in_=ps[:])
                fa = tmp.tile([MP, W2], BF16, tag="fa", name=f"fa{sg}")
                nc.vector.tensor_scalar_mul(
                    out=fa[:], in0=ft[:], scalar1=avec_sb[:, 0:1])

                qt = tmp.tile([MP, W2], BF16, tag="q", name=f"q{sg}")
                nc.vector.tensor_add(out=qt[:], in0=fa[:], in1=tq[:])
                pt = tmp.tile([MP, W2], BF16, tag="prod", name=f"prod{sg}")
                nc.vector.tensor_mul(out=pt[:], in0=ft[:], in1=qt[:])
                ot = io.tile([MP, W2], BF16, tag="o", name=f"o{sg}")
                nc.vector.tensor_sub(out=ot[:], in0=rt[:], in1=pt[:])
                nc.gpsimd.dma_start(out=out_p[sg], in_=ot[:, :])

    nc.compile()
    return nc


def _shard_inputs(rgb, d, rgb_var, d_var, W_prob, W_unc, W_total):
    rgb = np.asarray(rgb, np.float32)
    d = np.asarray(d, np.float32)
    rgb_var = np.asarray(rgb_var, np.float32)
    d_var = np.asarray(d_var, np.float32)
    b49, dmat, avec = _build_mats(
        np.asarray(W_prob, np.float32),
        np.asarray(W_unc, np.float32),
        np.asarray(W_total, np.float32))
    diff = rgb - d
    in_maps = []
    for core in range(NCORES):
        b, half = divmod(core, 2)
        h0 = half * R
        in_maps.append({
            "rgb_p": _pack_rows(rgb[b, :, h0:h0 + R, :]),
            "diff_p": _pack_rows(diff[b, :, h0:h0 + R, :]),
            "var_p": _pack_vars(rgb_var, d_var, b, h0),
            "b49": b49, "dmat": dmat, "avec": avec,
        })
    return in_maps


def run(trace=False, **inputs):
    if "nc" not in _CACHE:
        _CACHE["nc"] = _build_program()
    nc = _CACHE["nc"]
    in_maps = _shard_inputs(**inputs)
    res = run_bass_kernel_spmd(nc, in_maps, list(range(NCORES)), trace=trace)
    out = np.empty((B, C, H, W), np.float32)
    for core in range(NCORES):
        b, half = divmod(core, 2)
        out[b, :, half * R:(half + 1) * R, :] = _unpack_rows(
            res.results[core]["out_p"])
    return out, res


def kernel(**inputs):
    out, _ = run(trace=False, **inputs)
    return out

